# revision 6
# baseline (speedup 1.0000x reference)
"""Causal single-head attention (B=8, T=2048, D=1024, HS=64) on 8 TRN2 NeuronCores.

Sharding: data-parallel over batch -- core b computes batch b end-to-end.
No collectives; outputs are concatenated on the host.

Per-core pipeline (all heavy matmuls fp32r @ 1 cy/row, N>=256):
  for each 128-row t-tile j:
    DMA x[j]  ->  PE-transpose to x^T  ->  QKV = x @ [Wq/8 | Wk | Wv | 0]
    (one fused matmul, N=256; softmax 1/sqrt(HS) folded into Wq/bq)
    DVE bias-add from PSUM (the zero-pad column + bias gives V' ones-column
    for free)  ->  PE-transpose Q,K chunks into persistent Q^T/K^T [64, 2048]
    after t-tile 4s+3: attention for q-super s (512 queries):
      S^T[k,q] = K-chunk @ Q^T-super      (PSUM, N=512)
      P^T = exp(S^T)                      (ScalarE; logits ~N(0,1), no max sub)
      causal mask on diagonal chunks      (GPSIMD affine_select, in-place)
      outT[65,512] += V'[k,65]^T @ P^T    (PSUM accumulate; row 64 = denom)
      epilogue: PE-transpose outT back, DVE reciprocal + scale, DMA out.
"""

import sys

if "/opt/trn_rl_repo" not in sys.path:
    sys.path.insert(0, "/opt/trn_rl_repo")

import os
from contextlib import ExitStack

import numpy as np

import concourse.bass as bass
import concourse.tile as tile
from concourse import bacc, mybir
from concourse.bass_utils import run_bass_kernel_spmd

B, T, D, HS = 8, 2048, 1024, 64
N_CORES = 8
F32 = mybir.dt.float32
F32R = mybir.dt.float32r

TT = 128            # t/k tile (partition dim)
NDT = D // TT       # 8 contraction chunks
NTT = T // TT       # 16 t-tiles
QS = 512            # q-super width (matmul free dim)
NQS = T // QS       # 4 q-supers
WCOLS = 256         # QKV weight free dim, padded 192 -> 256 for fp32r speed
VP = HS + 1         # V' width (64 + ones column)


def build_graph() -> bacc.Bacc:
    nc = bacc.Bacc("TRN2", target_bir_lowering=False, debug=False)

    x_ext = nc.dram_tensor("x", [T, D], F32, kind="ExternalInput").ap()
    w_ext = nc.dram_tensor("wqkv", [D, WCOLS], F32, kind="ExternalInput").ap()
    bqk_ext = nc.dram_tensor("bqk", [TT, TT], F32, kind="ExternalInput").ap()
    bv1_ext = nc.dram_tensor("bv1", [TT, VP], F32, kind="ExternalInput").ap()
    id_ext = nc.dram_tensor("ident", [TT, TT], F32, kind="ExternalInput").ap()
    out_ext = nc.dram_tensor("out", [T, HS], F32, kind="ExternalOutput").ap()

    with tile.TileContext(nc) as tc, ExitStack() as ctx:
        const = ctx.enter_context(tc.tile_pool(name="const", bufs=1))
        persist = ctx.enter_context(tc.tile_pool(name="persist", bufs=1))
        xin_pool = ctx.enter_context(tc.tile_pool(name="xin", bufs=4))
        xt_pool = ctx.enter_context(tc.tile_pool(name="xt", bufs=2))
        qk_pool = ctx.enter_context(tc.tile_pool(name="qknat", bufs=2))
        pt_pool = ctx.enter_context(tc.tile_pool(name="pt", bufs=4))
        otsb_pool = ctx.enter_context(tc.tile_pool(name="otsb", bufs=2))
        osb_pool = ctx.enter_context(tc.tile_pool(name="osb", bufs=2))
        rc_pool = ctx.enter_context(tc.tile_pool(name="rc", bufs=2))
        psum = ctx.enter_context(tc.tile_pool(name="ps", bufs=1, space="PSUM"))

        # ---- constants ----
        w_sb = const.tile([TT, NDT * WCOLS], F32R)
        nc.sync.dma_start(
            w_sb[:].rearrange("p (c n) -> p c n", c=NDT),
            w_ext.rearrange("(c p) n -> p c n", p=TT).bitcast(F32R),
        )
        bqk_sb = const.tile([TT, TT], F32)
        nc.sync.dma_start(bqk_sb[:], bqk_ext)
        bv1_sb = const.tile([TT, VP], F32)
        nc.sync.dma_start(bv1_sb[:], bv1_ext)
        id_sb = const.tile([TT, TT], F32)
        nc.sync.dma_start(id_sb[:], id_ext)

        # ---- persistent per-core intermediates ----
        qt_sb = persist.tile([HS, T], F32R)      # Q^T / 8 (scale folded in W)
        kt_sb = persist.tile([HS, T], F32R)      # K^T
        vp_sb = persist.tile([TT, NTT * VP], F32R)  # V' tiles [128, 65] per k-tile

        def project_tile(j: int):
            """t-tile j: load x, transpose, QKV projection, fill QT/KT/V'."""
            x_nat = xin_pool.tile([TT, D], F32, tag="xnat")
            nc.sync.dma_start(x_nat[:], x_ext[j * TT:(j + 1) * TT, :])

            xt_sb = xt_pool.tile([TT, D], F32R, tag="xt")
            for g in range(2):
                txp = psum.tile([TT, 512], F32, tag="big", bufs=4)
                for b in range(4):
                    d = 4 * g + b
                    nc.tensor.transpose(
                        txp[:, b * TT:(b + 1) * TT],
                        x_nat[:, d * TT:(d + 1) * TT],
                        id_sb[:],
                    )
                nc.vector.tensor_copy(xt_sb[:, g * 512:(g + 1) * 512], txp[:])

            pj = psum.tile([TT, WCOLS], F32, tag="small", bufs=3)
            for d in range(NDT):
                nc.tensor.matmul(
                    pj[:],
                    xt_sb[:, d * TT:(d + 1) * TT],
                    w_sb[:, d * WCOLS:(d + 1) * WCOLS],
                    start=(d == 0),
                    stop=(d == NDT - 1),
                )

            qk_nat = qk_pool.tile([TT, TT], F32, tag="qknat")
            nc.vector.tensor_add(qk_nat[:], pj[:, 0:TT], bqk_sb[:])
            # V' = psum[:, 128:193] + [bv | 1.0]; col 192 of psum is exact 0
            nc.vector.tensor_add(
                vp_sb[:, j * VP:(j + 1) * VP], pj[:, TT:TT + VP], bv1_sb[:]
            )

            qt_ps = psum.tile([HS, 2 * TT], F32, tag="small", bufs=3)
            nc.tensor.transpose(qt_ps[:, 0:TT], qk_nat[:, 0:HS], id_sb[:])
            nc.tensor.transpose(qt_ps[:, TT:2 * TT], qk_nat[:, HS:2 * HS], id_sb[:])
            nc.vector.tensor_copy(qt_sb[:, j * TT:(j + 1) * TT], qt_ps[:, 0:TT])
            nc.vector.tensor_copy(kt_sb[:, j * TT:(j + 1) * TT], qt_ps[:, TT:2 * TT])

        def attend_super(s: int):
            """Causal attention for queries [512s, 512s+512); k-tiles 0..4s+3."""
            nkt = 4 * s + 4
            ot_ps = psum.tile([VP, QS], F32, tag="acc", bufs=1)
            for jj in range(nkt):
                s_ps = psum.tile([TT, QS], F32, tag="big", bufs=4)
                nc.tensor.matmul(
                    s_ps[:],
                    kt_sb[:, jj * TT:(jj + 1) * TT],
                    qt_sb[:, s * QS:(s + 1) * QS],
                    start=True,
                    stop=True,
                )
                ptile = pt_pool.tile([TT, QS], F32R, tag="pt")
                nc.scalar.activation(
                    ptile[:], s_ps[:], mybir.ActivationFunctionType.Exp
                )
                if jj >= 4 * s:
                    # diagonal chunk: zero P^T[kk, qq] where qq < kk + 128*dd
                    dd = jj - 4 * s
                    nc.gpsimd.affine_select(
                        out=ptile[:],
                        in_=ptile[:],
                        compare_op=mybir.AluOpType.is_ge,
                        fill=0.0,
                        base=-TT * dd,
                        channel_multiplier=-1,
                        pattern=[[1, QS]],
                    )
                nc.tensor.matmul(
                    ot_ps[:],
                    vp_sb[:, jj * VP:(jj + 1) * VP],
                    ptile[:],
                    start=(jj == 0),
                    stop=(jj == nkt - 1),
                    skip_group_check=True,
                )

            ot_sb = otsb_pool.tile([VP, QS], F32, tag="otsb")
            nc.scalar.copy(ot_sb[:], ot_ps[:])
            o_sb = osb_pool.tile([TT, 4 * HS], F32, tag="osb")
            for u in range(4):
                ob_ps = psum.tile([TT, VP], F32, tag="small", bufs=3)
                nc.tensor.transpose(
                    ob_ps[:], ot_sb[:, u * TT:(u + 1) * TT], id_sb[0:VP, 0:VP]
                )
                rc = rc_pool.tile([TT, 1], F32, tag="rc")
                nc.vector.reciprocal(rc[:], ob_ps[:, HS:HS + 1])
                nc.vector.tensor_scalar_mul(
                    o_sb[:, u * HS:(u + 1) * HS], ob_ps[:, 0:HS], rc[:]
                )
            nc.sync.dma_start(
                out_ext[s * QS:(s + 1) * QS, :].rearrange("(u p) h -> p u h", p=TT),
                o_sb[:].rearrange("p (u h) -> p u h", u=4),
            )

        for j in range(NTT):
            project_tile(j)
            if (j + 1) % 4 == 0:
                attend_super((j + 1) // 4 - 1)

    nc.compile()
    return nc


def make_inputs(x_b, Wq, bq, Wk, bk, Wv, bv):
    """Host-side constant prep for one core's in_map (x_b: [T, D])."""
    scale = 1.0 / np.sqrt(np.float32(HS))
    w_pad = np.zeros((D, WCOLS), dtype=np.float32)
    w_pad[:, 0:HS] = Wq * scale
    w_pad[:, HS:2 * HS] = Wk
    w_pad[:, 2 * HS:3 * HS] = Wv
    bqk = np.zeros((TT, TT), dtype=np.float32)
    bqk[:, 0:HS] = bq * scale
    bqk[:, HS:2 * HS] = bk
    bv1 = np.zeros((TT, VP), dtype=np.float32)
    bv1[:, 0:HS] = bv
    bv1[:, HS] = 1.0
    ident = np.eye(TT, dtype=np.float32)
    return {
        "x": np.ascontiguousarray(x_b, dtype=np.float32),
        "wqkv": w_pad,
        "bqk": bqk,
        "bv1": bv1,
        "ident": ident,
    }


_NC_CACHE = None


def _get_nc():
    global _NC_CACHE
    if _NC_CACHE is None:
        _NC_CACHE = build_graph()
    return _NC_CACHE


def kernel(x, Wq, bq, Wk, bk, Wv, bv):
    x = np.asarray(x, dtype=np.float32)
    args = [np.asarray(a, dtype=np.float32) for a in (Wq, bq, Wk, bk, Wv, bv)]
    nc = _get_nc()
    in_maps = [make_inputs(x[b], *args) for b in range(N_CORES)]
    trace = os.environ.get("BASS_ATTN_TRACE", "0") == "1"
    res = run_bass_kernel_spmd(
        nc, in_maps, core_ids=list(range(N_CORES)), trace=trace
    )
    if trace:
        print(
            f"HW exec time: {res.exec_time_ns} ns "
            f"(mean {res.mean_exec_time_ns}, max core {res.max_exec_time_core_id})"
        )
    out = np.stack([res.results[b]["out"] for b in range(N_CORES)], axis=0)
    return out


# revision 7
# speedup vs baseline: 1.0971x; 1.0971x over previous
"""Causal single-head attention (B=8, T=2048, D=1024, HS=64) on 8 TRN2 NeuronCores.

Sharding: data-parallel over batch -- core b computes batch b end-to-end.
No collectives; outputs are concatenated on the host.

Per-core pipeline, processed in four 512-row t-supers (all heavy matmuls
fp32r @ 1 cy/row, N=512):
  super ts:
    DMA x rows [512ts, 512ts+512) -> PE-transpose 32 [128,128] blocks to x^T
    projection with W stationary:  QKV^T[:,t] = W^T x^T  (two 128-col halves:
    [Wq/8 | Wk] and [Wv | 0]; softmax 1/sqrt(HS) folded into Wq/bq)
    DVE per-partition bias add straight from PSUM -> persistent Q^T/K^T [64,T]
    V^T + bias -> SBUF [65,512] with a ones-row -> PE-transpose to V' [128,65]
    attention (queries of this super, k-tiles 0..4ts+3), pipelined S(jj+1)
    ahead of PV(jj) so the PE never stalls on the exp chain:
      S^T[k,q] = K-chunk @ Q^T-super      (PSUM, N=512)
      P^T = exp(S^T)                      (ScalarE; logits ~N(0,1), no max sub)
      causal mask on diagonal chunks      (GPSIMD affine_select, in-place)
      outT[65,512] += V'[k,65]^T @ P^T    (PSUM accumulate; row 64 = denom)
    epilogue: PE-transpose outT back, DVE reciprocal + scale, DMA out.
"""

import sys

if "/opt/trn_rl_repo" not in sys.path:
    sys.path.insert(0, "/opt/trn_rl_repo")

import os
from contextlib import ExitStack

import numpy as np

import concourse.bass as bass
import concourse.tile as tile
from concourse import bacc, mybir
from concourse.bass_utils import run_bass_kernel_spmd

B, T, D, HS = 8, 2048, 1024, 64
N_CORES = 8
F32 = mybir.dt.float32
F32R = mybir.dt.float32r

TT = 128            # t/k tile (partition dim)
NDT = D // TT       # 8 contraction chunks
NTT = T // TT       # 16 k-tiles
QS = 512            # t/q super width (matmul free dim)
NQS = T // QS       # 4 supers
VP = HS + 1         # V' width (64 + ones column)


def build_graph() -> bacc.Bacc:
    nc = bacc.Bacc("TRN2", target_bir_lowering=False, debug=False)

    x_ext = nc.dram_tensor("x", [T, D], F32, kind="ExternalInput").ap()
    # wqkv[d, 0:128] = [Wq/8 | Wk]; wqkv[d, 128:256] = [Wv | 0]
    w_ext = nc.dram_tensor("wqkv", [D, 2 * TT], F32, kind="ExternalInput").ap()
    # bias columns: bcol[0:64,0] = bq/8, bcol[64:128,0] = bk, bcol[0:64,1] = bv
    bcol_ext = nc.dram_tensor("bcol", [TT, 2], F32, kind="ExternalInput").ap()
    id_ext = nc.dram_tensor("ident", [TT, TT], F32, kind="ExternalInput").ap()
    out_ext = nc.dram_tensor("out", [T, HS], F32, kind="ExternalOutput").ap()

    with tile.TileContext(nc) as tc, ExitStack() as ctx:
        const = ctx.enter_context(tc.tile_pool(name="const", bufs=1))
        persist = ctx.enter_context(tc.tile_pool(name="persist", bufs=1))
        xin_pool = ctx.enter_context(tc.tile_pool(name="xin", bufs=2))
        xt_pool = ctx.enter_context(tc.tile_pool(name="xt", bufs=2))
        vt_pool = ctx.enter_context(tc.tile_pool(name="vt", bufs=2))
        pt_pool = ctx.enter_context(tc.tile_pool(name="pt", bufs=4))
        otsb_pool = ctx.enter_context(tc.tile_pool(name="otsb", bufs=2))
        osb_pool = ctx.enter_context(tc.tile_pool(name="osb", bufs=2))
        rc_pool = ctx.enter_context(tc.tile_pool(name="rc", bufs=2))
        psum = ctx.enter_context(tc.tile_pool(name="ps", bufs=1, space="PSUM"))

        # ---- constants (scalar-engine HWDGE queue; x loads use sync queue) ----
        id_sb = const.tile([TT, TT], F32)
        nc.scalar.dma_start(id_sb[:], id_ext)
        w_sb = const.tile([TT, NDT * 2 * TT], F32R)
        nc.scalar.dma_start(
            w_sb[:].rearrange("p (c n) -> p c n", c=NDT),
            w_ext.rearrange("(c p) n -> p c n", p=TT).bitcast(F32R),
        )
        bcol_sb = const.tile([TT, 2], F32)
        nc.scalar.dma_start(bcol_sb[:], bcol_ext)

        # ---- persistent per-core intermediates ----
        qt_sb = persist.tile([HS, T], F32R)         # Q^T / 8 (scale folded)
        kt_sb = persist.tile([HS, T], F32R)         # K^T
        vp_sb = persist.tile([TT, NTT * VP], F32R)  # V' [128, 65] per k-tile

        def do_super(ts: int):
            tsl = slice(ts * QS, (ts + 1) * QS)

            # -- load x rows of this super --
            xn = xin_pool.tile([TT, 4 * D], F32, tag="xn")
            nc.sync.dma_start(
                xn[:].rearrange("p (u d) -> p u d", u=4),
                x_ext[tsl, :].rearrange("(u p) d -> p u d", p=TT),
            )

            # -- transpose to x^T: xt[:, 512c:(c+1)512] = x^T chunk c --
            xt = xt_pool.tile([TT, NDT * QS], F32R, tag="xt")
            for c in range(NDT):
                txp = psum.tile([TT, QS], F32, tag="big", bufs=4)
                for u in range(4):
                    nc.tensor.transpose(
                        txp[:, u * TT:(u + 1) * TT],
                        xn[:, u * D + c * TT:u * D + (c + 1) * TT],
                        id_sb[:],
                    )
                # alternate copy engine to split the PSUM->SBUF bandwidth
                if c % 2 == 0:
                    nc.vector.tensor_copy(xt[:, c * QS:(c + 1) * QS], txp[:])
                else:
                    nc.scalar.copy(xt[:, c * QS:(c + 1) * QS], txp[:])

            # -- projection, W stationary: psum = W_half^T @ x^T --
            for half in range(2):
                pp = psum.tile([TT, QS], F32, tag="big", bufs=4)
                for c in range(NDT):
                    nc.tensor.matmul(
                        pp[:],
                        w_sb[:, c * 2 * TT + half * TT:c * 2 * TT + (half + 1) * TT],
                        xt[:, c * QS:(c + 1) * QS],
                        start=(c == 0),
                        stop=(c == NDT - 1),
                    )
                if half == 0:
                    # rows 0:64 = Q^T/8, rows 64:128 = K^T (per-partition bias)
                    nc.vector.tensor_scalar_add(
                        qt_sb[:, tsl], pp[0:HS, :], bcol_sb[0:HS, 0:1]
                    )
                    nc.vector.tensor_scalar_add(
                        kt_sb[:, tsl], pp[HS:2 * HS, :], bcol_sb[HS:2 * HS, 0:1]
                    )
                else:
                    vt = vt_pool.tile([VP, QS], F32, tag="vt")
                    nc.vector.tensor_scalar_add(
                        vt[0:HS, :], pp[0:HS, :], bcol_sb[0:HS, 1:2]
                    )
                    nc.gpsimd.memset(vt[HS:VP, :], 1.0)
                    for u in range(4):
                        j = 4 * ts + u
                        vps = psum.tile([TT, VP], F32, tag="small", bufs=2)
                        nc.tensor.transpose(
                            vps[:], vt[:, u * TT:(u + 1) * TT], id_sb[0:VP, 0:VP]
                        )
                        nc.vector.tensor_copy(
                            vp_sb[:, j * VP:(j + 1) * VP], vps[:]
                        )

            # -- causal attention for this super's queries --
            nkt = 4 * ts + 4
            ot_ps = psum.tile([VP, QS], F32, tag="acc", bufs=2)

            def emit_s(jj):
                s_ps = psum.tile([TT, QS], F32, tag="big", bufs=4, name=f"s_ps{jj}")
                nc.tensor.matmul(
                    s_ps[:],
                    kt_sb[:, jj * TT:(jj + 1) * TT],
                    qt_sb[:, tsl],
                    start=True,
                    stop=True,
                )
                ptile = pt_pool.tile([TT, QS], F32R, tag="pt", name=f"pt{jj}")
                nc.scalar.activation(
                    ptile[:], s_ps[:], mybir.ActivationFunctionType.Exp
                )
                if jj >= 4 * ts:
                    # diagonal chunk: zero P^T[kk, qq] where qq < kk + 128*dd
                    dd = jj - 4 * ts
                    nc.gpsimd.affine_select(
                        out=ptile[:],
                        in_=ptile[:],
                        compare_op=mybir.AluOpType.is_ge,
                        fill=0.0,
                        base=-TT * dd,
                        channel_multiplier=-1,
                        pattern=[[1, QS]],
                    )
                return ptile

            def emit_pv(jj, ptile):
                nc.tensor.matmul(
                    ot_ps[:],
                    vp_sb[:, jj * VP:(jj + 1) * VP],
                    ptile[:],
                    start=(jj == 0),
                    stop=(jj == nkt - 1),
                    skip_group_check=True,
                )

            # software pipeline: keep two S tiles in flight ahead of each PV
            ptiles = [emit_s(0)]
            if nkt > 1:
                ptiles.append(emit_s(1))
            for jj in range(nkt):
                if jj + 2 < nkt:
                    ptiles.append(emit_s(jj + 2))
                emit_pv(jj, ptiles[jj])

            # -- epilogue: normalize + transpose back + store --
            ot_sb = otsb_pool.tile([VP, QS], F32, tag="otsb")
            nc.scalar.copy(ot_sb[:], ot_ps[:])
            o_sb = osb_pool.tile([TT, 4 * HS], F32, tag="osb")
            for u in range(4):
                ob_ps = psum.tile([TT, VP], F32, tag="small", bufs=2)
                nc.tensor.transpose(
                    ob_ps[:], ot_sb[:, u * TT:(u + 1) * TT], id_sb[0:VP, 0:VP]
                )
                rc = rc_pool.tile([TT, 1], F32, tag="rc")
                nc.vector.reciprocal(rc[:], ob_ps[:, HS:HS + 1])
                nc.vector.tensor_scalar_mul(
                    o_sb[:, u * HS:(u + 1) * HS], ob_ps[:, 0:HS], rc[:]
                )
            nc.sync.dma_start(
                out_ext[tsl, :].rearrange("(u p) h -> p u h", p=TT),
                o_sb[:].rearrange("p (u h) -> p u h", u=4),
            )

        for ts in range(NQS):
            do_super(ts)

    nc.compile()
    return nc


def make_inputs(x_b, Wq, bq, Wk, bk, Wv, bv):
    """Host-side constant prep for one core's in_map (x_b: [T, D])."""
    scale = 1.0 / np.sqrt(np.float32(HS))
    w = np.zeros((D, 2 * TT), dtype=np.float32)
    w[:, 0:HS] = Wq * scale
    w[:, HS:2 * HS] = Wk
    w[:, 2 * HS:3 * HS] = Wv
    bcol = np.zeros((TT, 2), dtype=np.float32)
    bcol[0:HS, 0] = bq * scale
    bcol[HS:2 * HS, 0] = bk
    bcol[0:HS, 1] = bv
    ident = np.eye(TT, dtype=np.float32)
    return {
        "x": np.ascontiguousarray(x_b, dtype=np.float32),
        "wqkv": w,
        "bcol": bcol,
        "ident": ident,
    }


_NC_CACHE = None


def _get_nc():
    global _NC_CACHE
    if _NC_CACHE is None:
        _NC_CACHE = build_graph()
    return _NC_CACHE


def kernel(x, Wq, bq, Wk, bk, Wv, bv):
    x = np.asarray(x, dtype=np.float32)
    args = [np.asarray(a, dtype=np.float32) for a in (Wq, bq, Wk, bk, Wv, bv)]
    nc = _get_nc()
    in_maps = [make_inputs(x[b], *args) for b in range(N_CORES)]
    trace = os.environ.get("BASS_ATTN_TRACE", "0") == "1"
    res = run_bass_kernel_spmd(
        nc, in_maps, core_ids=list(range(N_CORES)), trace=trace
    )
    if trace:
        print(
            f"HW exec time: {res.exec_time_ns} ns "
            f"(mean {res.mean_exec_time_ns}, max core {res.max_exec_time_core_id})"
        )
    out = np.stack([res.results[b]["out"] for b in range(N_CORES)], axis=0)
    return out


# revision 11
# speedup vs baseline: 1.1765x; 1.0723x over previous
"""Causal single-head attention (B=8, T=2048, D=1024, HS=64) on 8 TRN2 NeuronCores.

Sharding: data-parallel over batch -- core b computes batch b end-to-end.
No collectives; outputs are concatenated on the host.

Per-core pipeline, processed in four 512-row t-supers (all heavy matmuls
fp32r @ 1 cy/row, N=512):
  super ts:
    DMA x rows (4x 128-row blocks) -> PE-transpose to x^T (bf16 identity:
    the moving operand is the identity, so transpose streams at 1 cy/row)
    projection with W stationary:  QKV^T[:,t] = W^T x^T  (two 128-col halves:
    [Wq/8 | Wk] and [Wv | 0]; softmax 1/sqrt(HS) folded into Wq/bq)
    DVE per-partition bias add straight from PSUM -> persistent Q^T/K^T [64,T]
    V^T + bias -> SBUF [65,512] with a ones-row -> PE-transpose to V' [128,65]
    attention (queries of this super, k-tiles 0..4ts+3) in PAIRS of k-tiles
    sharing a 2-bank PSUM tile (one exp per pair), S emitted a pair ahead of
    PV so the PE never stalls on the exp chain:
      S^T[k,q] = K-chunk @ Q^T-super      (PSUM, N=512)
      P^T = exp(S^T)                      (ScalarE; logits ~N(0,1), no max sub)
      causal mask on diagonal chunks      (GPSIMD affine_select, in-place)
      outT[65,512] += V'[k,65]^T @ P^T    (PSUM accumulate; row 64 = denom)
    epilogue: PE-transpose outT back, DVE reciprocal + scale, DMA out.
"""

import sys

if "/opt/trn_rl_repo" not in sys.path:
    sys.path.insert(0, "/opt/trn_rl_repo")

import os
from contextlib import ExitStack

import numpy as np

import concourse.bass as bass
import concourse.tile as tile
from concourse import bacc, mybir
from concourse.bass_utils import run_bass_kernel_spmd

B, T, D, HS = 8, 2048, 1024, 64
N_CORES = 8
F32 = mybir.dt.float32
F32R = mybir.dt.float32r
BF16 = mybir.dt.bfloat16

TT = 128            # t/k tile (partition dim)
NDT = D // TT       # 8 contraction chunks
NTT = T // TT       # 16 k-tiles
QS = 512            # t/q super width (matmul free dim)
NQS = T // QS       # 4 supers
VP = HS + 1         # V' width (64 + ones column)


def build_graph() -> bacc.Bacc:
    nc = bacc.Bacc("TRN2", target_bir_lowering=False, debug=False)

    x_ext = nc.dram_tensor("x", [T, D], F32, kind="ExternalInput").ap()
    # wqkv[d, 0:128] = [Wq/8 | Wk]; wqkv[d, 128:256] = [Wv | 0]
    w_ext = nc.dram_tensor("wqkv", [D, 2 * TT], F32, kind="ExternalInput").ap()
    # bias columns: bcol[0:64,0] = bq/8, bcol[64:128,0] = bk, bcol[0:64,1] = bv
    bcol_ext = nc.dram_tensor("bcol", [TT, 2], F32, kind="ExternalInput").ap()
    id_ext = nc.dram_tensor("ident", [TT, TT], F32, kind="ExternalInput").ap()
    out_ext = nc.dram_tensor("out", [T, HS], F32, kind="ExternalOutput").ap()

    with tile.TileContext(nc) as tc, ExitStack() as ctx:
        const = ctx.enter_context(tc.tile_pool(name="const", bufs=1))
        persist = ctx.enter_context(tc.tile_pool(name="persist", bufs=1))
        xin_pool = ctx.enter_context(tc.tile_pool(name="xin", bufs=2))
        xt_pool = ctx.enter_context(tc.tile_pool(name="xt", bufs=2))
        vt_pool = ctx.enter_context(tc.tile_pool(name="vt", bufs=2))
        pt_pool = ctx.enter_context(tc.tile_pool(name="pt", bufs=3))
        otsb_pool = ctx.enter_context(tc.tile_pool(name="otsb", bufs=2))
        osb_pool = ctx.enter_context(tc.tile_pool(name="osb", bufs=2))
        rc_pool = ctx.enter_context(tc.tile_pool(name="rc", bufs=2))
        psum = ctx.enter_context(tc.tile_pool(name="ps", bufs=1, space="PSUM"))

        # ---- constants (scalar-engine HWDGE queue; x loads use sync queue) ----
        id_sb = const.tile([TT, TT], F32)
        nc.scalar.dma_start(id_sb[:], id_ext)
        idr_sb = const.tile([TT, TT], F32R)
        nc.scalar.dma_start(idr_sb[:], id_ext.bitcast(F32R))
        bcol_sb = const.tile([TT, 2], F32)
        nc.scalar.dma_start(bcol_sb[:], bcol_ext)
        w_sb = const.tile([TT, NDT * 2 * TT], F32R)
        nc.scalar.dma_start(
            w_sb[:].rearrange("p (c n) -> p c n", c=NDT),
            w_ext.rearrange("(c p) n -> p c n", p=TT).bitcast(F32R),
        )

        # ---- persistent per-core intermediates ----
        qt_sb = persist.tile([HS, T], F32R)         # Q^T / 8 (scale folded)
        kt_sb = persist.tile([HS, T], F32R)         # K^T
        vp_sb = persist.tile([TT, NTT * VP], F32R)  # V' [128, 65] per k-tile

        def do_super(ts: int):
            tsl = slice(ts * QS, (ts + 1) * QS)

            # -- load x rows of this super: 4 DMAs so transposes start early --
            xn = xin_pool.tile([TT, 4 * D], F32R, tag="xn")
            for u in range(4):
                nc.sync.dma_start(
                    xn[:, u * D:(u + 1) * D],
                    x_ext[ts * QS + u * TT:ts * QS + (u + 1) * TT, :].bitcast(F32R),
                )

            # -- transpose to x^T in 2-chunk pairs (one copy per 2-bank tile) --
            xt = xt_pool.tile([TT, NDT * QS], F32R, tag="xt")
            for cp in range(NDT // 2):
                txp = psum.tile([TT, 2 * QS], F32R, tag="sbig", bufs=2)
                for h in range(2):
                    c = 2 * cp + h
                    for u in range(4):
                        nc.tensor.transpose(
                            txp[:, h * QS + u * TT:h * QS + (u + 1) * TT],
                            xn[:, u * D + c * TT:u * D + (c + 1) * TT],
                            idr_sb[:],
                        )
                if cp % 2 == 0:
                    nc.vector.tensor_copy(
                        xt[:, 2 * cp * QS:(2 * cp + 2) * QS], txp[:]
                    )
                else:
                    nc.scalar.copy(
                        xt[:, 2 * cp * QS:(2 * cp + 2) * QS], txp[:]
                    )

            # -- projection, W stationary: psum = W_half^T @ x^T --
            for half in range(2):
                pp = psum.tile([TT, QS], F32, tag="proj", bufs=1)
                for c in range(NDT):
                    nc.tensor.matmul(
                        pp[:],
                        w_sb[:, c * 2 * TT + half * TT:c * 2 * TT + (half + 1) * TT],
                        xt[:, c * QS:(c + 1) * QS],
                        start=(c == 0),
                        stop=(c == NDT - 1),
                    )
                if half == 0:
                    # rows 0:64 = Q^T/8, rows 64:128 = K^T (per-partition bias)
                    nc.vector.tensor_scalar_add(
                        qt_sb[:, tsl], pp[0:HS, :], bcol_sb[0:HS, 0:1]
                    )
                    nc.vector.tensor_scalar_add(
                        kt_sb[:, tsl], pp[HS:2 * HS, :], bcol_sb[HS:2 * HS, 0:1]
                    )
                else:
                    vt = vt_pool.tile([VP, QS], F32, tag="vt")
                    nc.vector.tensor_scalar_add(
                        vt[0:HS, :], pp[0:HS, :], bcol_sb[0:HS, 1:2]
                    )
                    nc.gpsimd.memset(vt[HS:VP, :], 1.0)
                    for u in range(4):
                        j = 4 * ts + u
                        vps = psum.tile([TT, VP], F32, tag="small", bufs=2)
                        nc.tensor.transpose(
                            vps[:], vt[:, u * TT:(u + 1) * TT], id_sb[0:VP, 0:VP]
                        )
                        nc.vector.tensor_copy(
                            vp_sb[:, j * VP:(j + 1) * VP], vps[:]
                        )

            # -- causal attention for this super's queries, k-tile PAIRS --
            nkt = 4 * ts + 4
            ot_ps = psum.tile([VP, QS], F32, tag="acc", bufs=1)

            def emit_s_pair(p):
                """S matmuls + one exp for k-tiles (2p, 2p+1); returns ptile."""
                sp = psum.tile([TT, 2 * QS], F32, tag="sbig", bufs=2, name=f"sp{p}")
                for h in range(2):
                    jj = 2 * p + h
                    nc.tensor.matmul(
                        sp[:, h * QS:(h + 1) * QS],
                        kt_sb[:, jj * TT:(jj + 1) * TT],
                        qt_sb[:, tsl],
                        start=True,
                        stop=True,
                    )
                ptile = pt_pool.tile([TT, 2 * QS], F32R, tag="pt", name=f"pt{p}")
                nc.scalar.activation(
                    ptile[:], sp[:], mybir.ActivationFunctionType.Exp
                )
                for h in range(2):
                    jj = 2 * p + h
                    if jj >= 4 * ts:
                        # diagonal: zero P^T[kk, qq] where qq < kk + 128*dd
                        dd = jj - 4 * ts
                        nc.gpsimd.affine_select(
                            out=ptile[:, h * QS:(h + 1) * QS],
                            in_=ptile[:, h * QS:(h + 1) * QS],
                            compare_op=mybir.AluOpType.is_ge,
                            fill=0.0,
                            base=-TT * dd,
                            channel_multiplier=-1,
                            pattern=[[1, QS]],
                        )
                return ptile

            def emit_pv_pair(p, ptile):
                for h in range(2):
                    jj = 2 * p + h
                    nc.tensor.matmul(
                        ot_ps[:],
                        vp_sb[:, jj * VP:(jj + 1) * VP],
                        ptile[:, h * QS:(h + 1) * QS],
                        start=(jj == 0),
                        stop=(jj == nkt - 1),
                        skip_group_check=True,
                    )

            npair = nkt // 2
            ptiles = [emit_s_pair(0)]
            for p in range(npair):
                if p + 1 < npair:
                    ptiles.append(emit_s_pair(p + 1))
                emit_pv_pair(p, ptiles[p])

            # -- epilogue: normalize + transpose back + store --
            ot_sb = otsb_pool.tile([VP, QS], F32, tag="otsb")
            nc.scalar.copy(ot_sb[:], ot_ps[:])
            o_sb = osb_pool.tile([TT, 4 * HS], F32, tag="osb")
            for u in range(4):
                ob_ps = psum.tile([TT, VP], F32, tag="small", bufs=2)
                nc.tensor.transpose(
                    ob_ps[:], ot_sb[:, u * TT:(u + 1) * TT], id_sb[0:VP, 0:VP]
                )
                rc = rc_pool.tile([TT, 1], F32, tag="rc")
                nc.vector.reciprocal(rc[:], ob_ps[:, HS:HS + 1])
                nc.vector.tensor_scalar_mul(
                    o_sb[:, u * HS:(u + 1) * HS], ob_ps[:, 0:HS], rc[:]
                )
            nc.sync.dma_start(
                out_ext[tsl, :].rearrange("(u p) h -> p u h", p=TT),
                o_sb[:].rearrange("p (u h) -> p u h", u=4),
            )

        for ts in range(NQS):
            do_super(ts)

    nc.compile()
    return nc


def make_inputs(x_b, Wq, bq, Wk, bk, Wv, bv):
    """Host-side constant prep for one core's in_map (x_b: [T, D])."""
    scale = 1.0 / np.sqrt(np.float32(HS))
    w = np.zeros((D, 2 * TT), dtype=np.float32)
    w[:, 0:HS] = Wq * scale
    w[:, HS:2 * HS] = Wk
    w[:, 2 * HS:3 * HS] = Wv
    bcol = np.zeros((TT, 2), dtype=np.float32)
    bcol[0:HS, 0] = bq * scale
    bcol[HS:2 * HS, 0] = bk
    bcol[0:HS, 1] = bv
    ident = np.eye(TT, dtype=np.float32)
    return {
        "x": np.ascontiguousarray(x_b, dtype=np.float32),
        "wqkv": w,
        "bcol": bcol,
        "ident": ident,
    }


_NC_CACHE = None


def _get_nc():
    global _NC_CACHE
    if _NC_CACHE is None:
        _NC_CACHE = build_graph()
    return _NC_CACHE


def kernel(x, Wq, bq, Wk, bk, Wv, bv):
    x = np.asarray(x, dtype=np.float32)
    args = [np.asarray(a, dtype=np.float32) for a in (Wq, bq, Wk, bk, Wv, bv)]
    nc = _get_nc()
    in_maps = [make_inputs(x[b], *args) for b in range(N_CORES)]
    trace = os.environ.get("BASS_ATTN_TRACE", "0") == "1"
    res = run_bass_kernel_spmd(
        nc, in_maps, core_ids=list(range(N_CORES)), trace=trace
    )
    if trace:
        print(
            f"HW exec time: {res.exec_time_ns} ns "
            f"(mean {res.mean_exec_time_ns}, max core {res.max_exec_time_core_id})"
        )
    out = np.stack([res.results[b]["out"] for b in range(N_CORES)], axis=0)
    return out


# revision 12
# speedup vs baseline: 1.1984x; 1.0187x over previous
"""Causal single-head attention (B=8, T=2048, D=1024, HS=64) on 8 TRN2 NeuronCores.

Sharding: data-parallel over batch -- core b computes batch b end-to-end.
No collectives; outputs are concatenated on the host.

Per-core pipeline, processed in four 512-row t-supers (all heavy matmuls
fp32r @ 1 cy/row, N=512):
  super ts:
    DMA x rows (4x 128-row blocks) -> PE-transpose to x^T (bf16 identity:
    the moving operand is the identity, so transpose streams at 1 cy/row)
    projection with W stationary:  QKV^T[:,t] = W^T x^T  (two 128-col halves:
    [Wq/8 | Wk] and [Wv | 0]; softmax 1/sqrt(HS) folded into Wq/bq)
    DVE per-partition bias add straight from PSUM -> persistent Q^T/K^T [64,T]
    V^T + bias -> SBUF [65,512] with a ones-row -> PE-transpose to V' [128,65]
    attention (queries of this super, k-tiles 0..4ts+3) in PAIRS of k-tiles
    sharing a 2-bank PSUM tile (one exp per pair), S emitted a pair ahead of
    PV so the PE never stalls on the exp chain:
      S^T[k,q] = K-chunk @ Q^T-super      (PSUM, N=512)
      P^T = exp(S^T)                      (ScalarE; logits ~N(0,1), no max sub)
      causal mask on diagonal chunks      (GPSIMD affine_select, in-place)
      outT[65,512] += V'[k,65]^T @ P^T    (PSUM accumulate; row 64 = denom)
    epilogue: PE-transpose outT back, DVE reciprocal + scale, DMA out.
"""

import sys

if "/opt/trn_rl_repo" not in sys.path:
    sys.path.insert(0, "/opt/trn_rl_repo")

import os
from contextlib import ExitStack

import numpy as np

import concourse.bass as bass
import concourse.tile as tile
from concourse import bacc, mybir
from concourse.bass_utils import run_bass_kernel_spmd

B, T, D, HS = 8, 2048, 1024, 64
N_CORES = 8
F32 = mybir.dt.float32
F32R = mybir.dt.float32r
BF16 = mybir.dt.bfloat16

TT = 128            # t/k tile (partition dim)
NDT = D // TT       # 8 contraction chunks
NTT = T // TT       # 16 k-tiles
QS = 512            # t/q super width (matmul free dim)
NQS = T // QS       # 4 supers
VP = HS + 1         # V' width (64 + ones column)


def build_graph() -> bacc.Bacc:
    nc = bacc.Bacc("TRN2", target_bir_lowering=False, debug=False)

    x_ext = nc.dram_tensor("x", [T, D], F32, kind="ExternalInput").ap()
    # wqkv[d, 0:128] = [Wq/8 | Wk]; wqkv[d, 128:256] = [Wv | 0]
    w_ext = nc.dram_tensor("wqkv", [D, 2 * TT], F32, kind="ExternalInput").ap()
    # bias columns: bcol[0:64,0] = bq/8, bcol[64:128,0] = bk, bcol[0:64,1] = bv
    bcol_ext = nc.dram_tensor("bcol", [TT, 2], F32, kind="ExternalInput").ap()
    id_ext = nc.dram_tensor("ident", [TT, TT], F32, kind="ExternalInput").ap()
    out_ext = nc.dram_tensor("out", [T, HS], F32, kind="ExternalOutput").ap()

    with tile.TileContext(nc) as tc, ExitStack() as ctx:
        const = ctx.enter_context(tc.tile_pool(name="const", bufs=1))
        persist = ctx.enter_context(tc.tile_pool(name="persist", bufs=1))
        xin_pool = ctx.enter_context(tc.tile_pool(name="xin", bufs=2))
        xt_pool = ctx.enter_context(tc.tile_pool(name="xt", bufs=2))
        vt_pool = ctx.enter_context(tc.tile_pool(name="vt", bufs=2))
        pt_pool = ctx.enter_context(tc.tile_pool(name="pt", bufs=3))
        otsb_pool = ctx.enter_context(tc.tile_pool(name="otsb", bufs=2))
        osb_pool = ctx.enter_context(tc.tile_pool(name="osb", bufs=2))
        rc_pool = ctx.enter_context(tc.tile_pool(name="rc", bufs=2))
        psum = ctx.enter_context(tc.tile_pool(name="ps", bufs=1, space="PSUM"))

        # ---- constants (scalar-engine HWDGE queue; x loads use sync queue) ----
        id_sb = const.tile([TT, TT], F32)
        nc.gpsimd.dma_start(id_sb[:], id_ext)
        idr_sb = const.tile([TT, TT], F32R)
        nc.gpsimd.dma_start(idr_sb[:], id_ext.bitcast(F32R))
        bcol_sb = const.tile([TT, 2], F32)
        nc.gpsimd.dma_start(bcol_sb[:], bcol_ext)
        w_sb = const.tile([TT, NDT * 2 * TT], F32R)
        nc.gpsimd.dma_start(
            w_sb[:].rearrange("p (c n) -> p c n", c=NDT),
            w_ext.rearrange("(c p) n -> p c n", p=TT).bitcast(F32R),
        )

        # ---- persistent per-core intermediates ----
        qt_sb = persist.tile([HS, T], F32R)         # Q^T / 8 (scale folded)
        kt_sb = persist.tile([HS, T], F32R)         # K^T
        vp_sb = persist.tile([TT, NTT * VP], F32R)  # V' [128, 65] per k-tile

        warm_pool = ctx.enter_context(tc.tile_pool(name="warm", bufs=1))
        warm_ps = psum.tile([TT, TT], F32, tag="small", bufs=2)
        for _ in range(10):
            nc.tensor.matmul(warm_ps[:], idr_sb[:], idr_sb[:], start=True, stop=True)

        def do_super(ts: int):
            tsl = slice(ts * QS, (ts + 1) * QS)

            # -- load x rows of this super: 4 DMAs so transposes start early --
            xn = xin_pool.tile([TT, 4 * D], F32R, tag="xn")
            for u in range(4):
                nc.sync.dma_start(
                    xn[:, u * D:(u + 1) * D],
                    x_ext[ts * QS + u * TT:ts * QS + (u + 1) * TT, :].bitcast(F32R),
                )

            # -- transpose to x^T in 2-chunk pairs (one copy per 2-bank tile) --
            xt = xt_pool.tile([TT, NDT * QS], F32R, tag="xt")
            for cp in range(NDT // 2):
                txp = psum.tile([TT, 2 * QS], F32R, tag="sbig", bufs=2)
                for h in range(2):
                    c = 2 * cp + h
                    for u in range(4):
                        nc.tensor.transpose(
                            txp[:, h * QS + u * TT:h * QS + (u + 1) * TT],
                            xn[:, u * D + c * TT:u * D + (c + 1) * TT],
                            idr_sb[:],
                        )
                if cp % 2 == 0:
                    nc.vector.tensor_copy(
                        xt[:, 2 * cp * QS:(2 * cp + 2) * QS], txp[:]
                    )
                else:
                    nc.scalar.copy(
                        xt[:, 2 * cp * QS:(2 * cp + 2) * QS], txp[:]
                    )

            # -- projection, W stationary: psum = W_half^T @ x^T --
            for half in range(2):
                pp = psum.tile([TT, QS], F32, tag="proj", bufs=1)
                for c in range(NDT):
                    nc.tensor.matmul(
                        pp[:],
                        w_sb[:, c * 2 * TT + half * TT:c * 2 * TT + (half + 1) * TT],
                        xt[:, c * QS:(c + 1) * QS],
                        start=(c == 0),
                        stop=(c == NDT - 1),
                    )
                if half == 0:
                    # rows 0:64 = Q^T/8, rows 64:128 = K^T (per-partition bias)
                    nc.vector.tensor_scalar_add(
                        qt_sb[:, tsl], pp[0:HS, :], bcol_sb[0:HS, 0:1]
                    )
                    nc.vector.tensor_scalar_add(
                        kt_sb[:, tsl], pp[HS:2 * HS, :], bcol_sb[HS:2 * HS, 0:1]
                    )
                else:
                    vt = vt_pool.tile([VP, QS], F32, tag="vt")
                    nc.vector.tensor_scalar_add(
                        vt[0:HS, :], pp[0:HS, :], bcol_sb[0:HS, 1:2]
                    )
                    nc.gpsimd.memset(vt[HS:VP, :], 1.0)
                    for u in range(4):
                        j = 4 * ts + u
                        vps = psum.tile([TT, VP], F32, tag="small", bufs=2)
                        nc.tensor.transpose(
                            vps[:], vt[:, u * TT:(u + 1) * TT], id_sb[0:VP, 0:VP]
                        )
                        nc.vector.tensor_copy(
                            vp_sb[:, j * VP:(j + 1) * VP], vps[:]
                        )

            # -- causal attention for this super's queries, k-tile PAIRS --
            nkt = 4 * ts + 4
            ot_ps = psum.tile([VP, QS], F32, tag="acc", bufs=1)

            def emit_s_pair(p):
                """S matmuls + one exp for k-tiles (2p, 2p+1); returns ptile."""
                sp = psum.tile([TT, 2 * QS], F32, tag="sbig", bufs=2, name=f"sp{p}")
                for h in range(2):
                    jj = 2 * p + h
                    nc.tensor.matmul(
                        sp[:, h * QS:(h + 1) * QS],
                        kt_sb[:, jj * TT:(jj + 1) * TT],
                        qt_sb[:, tsl],
                        start=True,
                        stop=True,
                    )
                ptile = pt_pool.tile([TT, 2 * QS], F32R, tag="pt", name=f"pt{p}")
                nc.scalar.activation(
                    ptile[:], sp[:], mybir.ActivationFunctionType.Exp
                )
                for h in range(2):
                    jj = 2 * p + h
                    if jj >= 4 * ts:
                        # diagonal: zero P^T[kk, qq] where qq < kk + 128*dd
                        dd = jj - 4 * ts
                        nc.gpsimd.affine_select(
                            out=ptile[:, h * QS:(h + 1) * QS],
                            in_=ptile[:, h * QS:(h + 1) * QS],
                            compare_op=mybir.AluOpType.is_ge,
                            fill=0.0,
                            base=-TT * dd,
                            channel_multiplier=-1,
                            pattern=[[1, QS]],
                        )
                return ptile

            def emit_pv_pair(p, ptile):
                for h in range(2):
                    jj = 2 * p + h
                    nc.tensor.matmul(
                        ot_ps[:],
                        vp_sb[:, jj * VP:(jj + 1) * VP],
                        ptile[:, h * QS:(h + 1) * QS],
                        start=(jj == 0),
                        stop=(jj == nkt - 1),
                        skip_group_check=True,
                    )

            npair = nkt // 2
            ptiles = [emit_s_pair(0)]
            for p in range(npair):
                if p + 1 < npair:
                    ptiles.append(emit_s_pair(p + 1))
                emit_pv_pair(p, ptiles[p])

            # -- epilogue: normalize + transpose back + store --
            ot_sb = otsb_pool.tile([VP, QS], F32, tag="otsb")
            nc.scalar.copy(ot_sb[:], ot_ps[:])
            o_sb = osb_pool.tile([TT, 4 * HS], F32, tag="osb")
            for u in range(4):
                ob_ps = psum.tile([TT, VP], F32, tag="small", bufs=2)
                nc.tensor.transpose(
                    ob_ps[:], ot_sb[:, u * TT:(u + 1) * TT], id_sb[0:VP, 0:VP]
                )
                rc = rc_pool.tile([TT, 1], F32, tag="rc")
                nc.vector.reciprocal(rc[:], ob_ps[:, HS:HS + 1])
                nc.vector.tensor_scalar_mul(
                    o_sb[:, u * HS:(u + 1) * HS], ob_ps[:, 0:HS], rc[:]
                )
            nc.sync.dma_start(
                out_ext[tsl, :].rearrange("(u p) h -> p u h", p=TT),
                o_sb[:].rearrange("p (u h) -> p u h", u=4),
            )

        for ts in range(NQS):
            do_super(ts)

    nc.compile()
    return nc


def make_inputs(x_b, Wq, bq, Wk, bk, Wv, bv):
    """Host-side constant prep for one core's in_map (x_b: [T, D])."""
    scale = 1.0 / np.sqrt(np.float32(HS))
    w = np.zeros((D, 2 * TT), dtype=np.float32)
    w[:, 0:HS] = Wq * scale
    w[:, HS:2 * HS] = Wk
    w[:, 2 * HS:3 * HS] = Wv
    bcol = np.zeros((TT, 2), dtype=np.float32)
    bcol[0:HS, 0] = bq * scale
    bcol[HS:2 * HS, 0] = bk
    bcol[0:HS, 1] = bv
    ident = np.eye(TT, dtype=np.float32)
    return {
        "x": np.ascontiguousarray(x_b, dtype=np.float32),
        "wqkv": w,
        "bcol": bcol,
        "ident": ident,
    }


_NC_CACHE = None


def _get_nc():
    global _NC_CACHE
    if _NC_CACHE is None:
        _NC_CACHE = build_graph()
    return _NC_CACHE


def kernel(x, Wq, bq, Wk, bk, Wv, bv):
    x = np.asarray(x, dtype=np.float32)
    args = [np.asarray(a, dtype=np.float32) for a in (Wq, bq, Wk, bk, Wv, bv)]
    nc = _get_nc()
    in_maps = [make_inputs(x[b], *args) for b in range(N_CORES)]
    trace = os.environ.get("BASS_ATTN_TRACE", "0") == "1"
    res = run_bass_kernel_spmd(
        nc, in_maps, core_ids=list(range(N_CORES)), trace=trace
    )
    if trace:
        print(
            f"HW exec time: {res.exec_time_ns} ns "
            f"(mean {res.mean_exec_time_ns}, max core {res.max_exec_time_core_id})"
        )
    out = np.stack([res.results[b]["out"] for b in range(N_CORES)], axis=0)
    return out


# revision 16
# speedup vs baseline: 1.2056x; 1.0060x over previous
"""Causal single-head attention (B=8, T=2048, D=1024, HS=64) on 8 TRN2 NeuronCores.

Sharding: data-parallel over batch -- core b computes batch b end-to-end.
No collectives; outputs are concatenated on the host.

Per-core pipeline, processed in four 512-row t-supers (all heavy matmuls
fp32r @ 1 cy/row, N=512):
  super ts:
    DMA x rows (4x 128-row blocks) -> PE-transpose to x^T (bf16 identity:
    the moving operand is the identity, so transpose streams at 1 cy/row)
    projection with W stationary:  QKV^T[:,t] = W^T x^T  (two 128-col halves:
    [Wq/8 | Wk] and [Wv | 0]; softmax 1/sqrt(HS) folded into Wq/bq)
    DVE per-partition bias add straight from PSUM -> persistent Q^T/K^T [64,T]
    V^T + bias -> SBUF [65,512] with a ones-row -> PE-transpose to V' [128,65]
    attention (queries of this super, k-tiles 0..4ts+3) in PAIRS of k-tiles
    sharing a 2-bank PSUM tile (one exp per pair), S emitted a pair ahead of
    PV so the PE never stalls on the exp chain:
      S^T[k,q] = K-chunk @ Q^T-super      (PSUM, N=512)
      P^T = exp(S^T)                      (ScalarE; logits ~N(0,1), no max sub)
      causal mask on diagonal chunks      (GPSIMD affine_select, in-place)
      outT[65,512] += V'[k,65]^T @ P^T    (PSUM accumulate; row 64 = denom)
    epilogue: PE-transpose outT back, DVE reciprocal + scale, DMA out.
"""

import sys

if "/opt/trn_rl_repo" not in sys.path:
    sys.path.insert(0, "/opt/trn_rl_repo")

import os
from contextlib import ExitStack

import numpy as np

import concourse.bass as bass
import concourse.tile as tile
from concourse import bacc, mybir
from concourse.bass_utils import run_bass_kernel_spmd

B, T, D, HS = 8, 2048, 1024, 64
N_CORES = 8
F32 = mybir.dt.float32
F32R = mybir.dt.float32r
BF16 = mybir.dt.bfloat16

TT = 128            # t/k tile (partition dim)
NDT = D // TT       # 8 contraction chunks
NTT = T // TT       # 16 k-tiles
QS = 512            # t/q super width (matmul free dim)
NQS = T // QS       # 4 supers
VP = HS + 1         # V' width (64 + ones column)


def build_graph() -> bacc.Bacc:
    nc = bacc.Bacc("TRN2", target_bir_lowering=False, debug=False)

    x_ext = nc.dram_tensor("x", [T, D], F32, kind="ExternalInput").ap()
    # wqkv[d, 0:128] = [Wq/8 | Wk]; wqkv[d, 128:256] = [Wv | 0]
    w_ext = nc.dram_tensor("wqkv", [D, 2 * TT], F32, kind="ExternalInput").ap()
    # bias columns: bcol[0:64,0] = bq/8, bcol[64:128,0] = bk, bcol[0:64,1] = bv
    bcol_ext = nc.dram_tensor("bcol", [TT, 2], F32, kind="ExternalInput").ap()
    id_ext = nc.dram_tensor("ident", [TT, TT], F32, kind="ExternalInput").ap()
    out_ext = nc.dram_tensor("out", [T, HS], F32, kind="ExternalOutput").ap()

    with tile.TileContext(nc) as tc, ExitStack() as ctx:
        const = ctx.enter_context(tc.tile_pool(name="const", bufs=1))
        persist = ctx.enter_context(tc.tile_pool(name="persist", bufs=1))
        xin_pool = ctx.enter_context(tc.tile_pool(name="xin", bufs=2))
        xt_pool = ctx.enter_context(tc.tile_pool(name="xt", bufs=2))
        vt_pool = ctx.enter_context(tc.tile_pool(name="vt", bufs=2))
        pt_pool = ctx.enter_context(tc.tile_pool(name="pt", bufs=3))
        otsb_pool = ctx.enter_context(tc.tile_pool(name="otsb", bufs=2))
        osb_pool = ctx.enter_context(tc.tile_pool(name="osb", bufs=2))
        rc_pool = ctx.enter_context(tc.tile_pool(name="rc", bufs=2))
        psum = ctx.enter_context(tc.tile_pool(name="ps", bufs=1, space="PSUM"))

        # ---- constants (scalar-engine HWDGE queue; x loads use sync queue) ----
        id_sb = const.tile([TT, TT], F32)
        nc.sync.dma_start(id_sb[:], id_ext)
        idr_sb = const.tile([TT, TT], F32R)
        nc.sync.dma_start(idr_sb[:], id_ext.bitcast(F32R))
        bcol_sb = const.tile([TT, 2], F32)
        nc.sync.dma_start(bcol_sb[:], bcol_ext)
        w_sb = const.tile([TT, NDT * 2 * TT], F32R)

        def load_w():
            nc.sync.dma_start(
                w_sb[:].rearrange("p (c n) -> p c n", c=NDT),
                w_ext.rearrange("(c p) n -> p c n", p=TT).bitcast(F32R),
            )

        # ---- persistent per-core intermediates ----
        qt_sb = persist.tile([HS, T], F32R)         # Q^T / 8 (scale folded)
        kt_sb = persist.tile([HS, T], F32R)         # K^T
        vp_sb = persist.tile([TT, NTT * VP], F32R)  # V' [128, 65] per k-tile

        warm_pool = ctx.enter_context(tc.tile_pool(name="warm", bufs=1))
        warm_sb = warm_pool.tile([TT, QS], F32)
        nc.gpsimd.memset(warm_sb[:], 0.0)
        for _ in range(4):
            warm_ps = psum.tile([TT, QS], F32, tag="proj", bufs=1)
            nc.tensor.matmul(
                warm_ps[:], warm_sb[:, 0:TT], warm_sb[:], start=True, stop=True
            )

        def keepwarm():
            # brief real matmul so the HAM activity monitor keeps the PE
            # clock at 2.4 GHz through transpose-only stretches
            wp = psum.tile([TT, TT], F32, tag="small", bufs=2, name="kw")
            nc.tensor.matmul(wp[:], warm_sb[:, 0:TT], warm_sb[:, 0:TT],
                             start=True, stop=True)

        def do_super(ts: int):
            tsl = slice(ts * QS, (ts + 1) * QS)

            # -- load x rows of this super: 4 DMAs so transposes start early --
            xn = xin_pool.tile([TT, 4 * D], F32R, tag="xn")
            for u in range(4):
                nc.sync.dma_start(
                    xn[:, u * D:(u + 1) * D],
                    x_ext[ts * QS + u * TT:ts * QS + (u + 1) * TT, :].bitcast(F32R),
                )

            if ts == 0:
                load_w()

            # -- transpose to x^T in 2-chunk pairs (one copy per 2-bank tile) --
            xt = xt_pool.tile([TT, NDT * QS], F32R, tag="xt")
            for cp in range(NDT // 2):
                txp = psum.tile([TT, 2 * QS], F32R, tag="sbig", bufs=2)
                for h in range(2):
                    c = 2 * cp + h
                    for u in range(4):
                        nc.tensor.transpose(
                            txp[:, h * QS + u * TT:h * QS + (u + 1) * TT],
                            xn[:, u * D + c * TT:u * D + (c + 1) * TT],
                            idr_sb[:],
                        )
                if cp % 2 == 0:
                    nc.vector.tensor_copy(
                        xt[:, 2 * cp * QS:(2 * cp + 2) * QS], txp[:]
                    )
                else:
                    nc.scalar.copy(
                        xt[:, 2 * cp * QS:(2 * cp + 2) * QS], txp[:]
                    )
                keepwarm()

            # -- projection, W stationary: psum = W_half^T @ x^T --
            for half in range(2):
                pp = psum.tile([TT, QS], F32, tag="proj", bufs=1)
                for c in range(NDT):
                    nc.tensor.matmul(
                        pp[:],
                        w_sb[:, c * 2 * TT + half * TT:c * 2 * TT + (half + 1) * TT],
                        xt[:, c * QS:(c + 1) * QS],
                        start=(c == 0),
                        stop=(c == NDT - 1),
                    )
                if half == 0:
                    # rows 0:64 = Q^T/8, rows 64:128 = K^T (per-partition bias)
                    nc.vector.tensor_scalar_add(
                        qt_sb[:, tsl], pp[0:HS, :], bcol_sb[0:HS, 0:1]
                    )
                    nc.vector.tensor_scalar_add(
                        kt_sb[:, tsl], pp[HS:2 * HS, :], bcol_sb[HS:2 * HS, 0:1]
                    )
                else:
                    # rows 0:64 = V^T + bv; row 64 = 0 (W zero-pad) + 1.0
                    vt = vt_pool.tile([VP, QS], F32, tag="vt")
                    nc.vector.tensor_scalar_add(
                        vt[0:VP, :], pp[0:VP, :], bcol_sb[0:VP, 1:2]
                    )
                    for u in range(4):
                        j = 4 * ts + u
                        vps = psum.tile([TT, VP], F32, tag="small", bufs=2)
                        nc.tensor.transpose(
                            vps[:], vt[:, u * TT:(u + 1) * TT], id_sb[0:VP, 0:VP]
                        )
                        nc.vector.tensor_copy(
                            vp_sb[:, j * VP:(j + 1) * VP], vps[:]
                        )

            # -- causal attention for this super's queries, k-tile PAIRS --
            nkt = 4 * ts + 4
            ot_ps = psum.tile([VP, QS], F32, tag="acc", bufs=1)

            def emit_s_pair(p):
                """S matmuls + one exp for k-tiles (2p, 2p+1); returns ptile."""
                sp = psum.tile([TT, 2 * QS], F32, tag="sbig", bufs=2, name=f"sp{p}")
                for h in range(2):
                    jj = 2 * p + h
                    nc.tensor.matmul(
                        sp[:, h * QS:(h + 1) * QS],
                        kt_sb[:, jj * TT:(jj + 1) * TT],
                        qt_sb[:, tsl],
                        start=True,
                        stop=True,
                    )
                ptile = pt_pool.tile([TT, 2 * QS], F32R, tag="pt", name=f"pt{p}")
                nc.scalar.activation(
                    ptile[:], sp[:], mybir.ActivationFunctionType.Exp
                )
                for h in range(2):
                    jj = 2 * p + h
                    if jj >= 4 * ts:
                        # diagonal: zero P^T[kk, qq] where qq < kk + 128*dd
                        dd = jj - 4 * ts
                        nc.gpsimd.affine_select(
                            out=ptile[:, h * QS:(h + 1) * QS],
                            in_=ptile[:, h * QS:(h + 1) * QS],
                            compare_op=mybir.AluOpType.is_ge,
                            fill=0.0,
                            base=-TT * dd,
                            channel_multiplier=-1,
                            pattern=[[1, QS]],
                        )
                return ptile

            def emit_pv_pair(p, ptile):
                for h in range(2):
                    jj = 2 * p + h
                    nc.tensor.matmul(
                        ot_ps[:],
                        vp_sb[:, jj * VP:(jj + 1) * VP],
                        ptile[:, h * QS:(h + 1) * QS],
                        start=(jj == 0),
                        stop=(jj == nkt - 1),
                        skip_group_check=True,
                    )

            npair = nkt // 2
            ptiles = [emit_s_pair(0)]
            for p in range(npair):
                if p + 1 < npair:
                    ptiles.append(emit_s_pair(p + 1))
                emit_pv_pair(p, ptiles[p])

            # -- epilogue: normalize + transpose back + store --
            ot_sb = otsb_pool.tile([VP, QS], F32, tag="otsb")
            nc.scalar.copy(ot_sb[:], ot_ps[:])
            o_sb = osb_pool.tile([TT, 4 * HS], F32, tag="osb")
            for u in range(4):
                ob_ps = psum.tile([TT, VP], F32, tag="small", bufs=2)
                nc.tensor.transpose(
                    ob_ps[:], ot_sb[:, u * TT:(u + 1) * TT], id_sb[0:VP, 0:VP]
                )
                rc = rc_pool.tile([TT, 1], F32, tag="rc")
                nc.vector.reciprocal(rc[:], ob_ps[:, HS:HS + 1])
                nc.vector.tensor_scalar_mul(
                    o_sb[:, u * HS:(u + 1) * HS], ob_ps[:, 0:HS], rc[:]
                )
            nc.sync.dma_start(
                out_ext[tsl, :].rearrange("(u p) h -> p u h", p=TT),
                o_sb[:].rearrange("p (u h) -> p u h", u=4),
            )

        for ts in range(NQS):
            do_super(ts)

    nc.compile()
    return nc


def make_inputs(x_b, Wq, bq, Wk, bk, Wv, bv):
    """Host-side constant prep for one core's in_map (x_b: [T, D])."""
    scale = 1.0 / np.sqrt(np.float32(HS))
    w = np.zeros((D, 2 * TT), dtype=np.float32)
    w[:, 0:HS] = Wq * scale
    w[:, HS:2 * HS] = Wk
    w[:, 2 * HS:3 * HS] = Wv
    bcol = np.zeros((TT, 2), dtype=np.float32)
    bcol[0:HS, 0] = bq * scale
    bcol[HS:2 * HS, 0] = bk
    bcol[0:HS, 1] = bv
    bcol[HS, 1] = 1.0
    ident = np.eye(TT, dtype=np.float32)
    return {
        "x": np.ascontiguousarray(x_b, dtype=np.float32),
        "wqkv": w,
        "bcol": bcol,
        "ident": ident,
    }


_NC_CACHE = None


def _get_nc():
    global _NC_CACHE
    if _NC_CACHE is None:
        _NC_CACHE = build_graph()
    return _NC_CACHE


def kernel(x, Wq, bq, Wk, bk, Wv, bv):
    x = np.asarray(x, dtype=np.float32)
    args = [np.asarray(a, dtype=np.float32) for a in (Wq, bq, Wk, bk, Wv, bv)]
    nc = _get_nc()
    in_maps = [make_inputs(x[b], *args) for b in range(N_CORES)]
    trace = os.environ.get("BASS_ATTN_TRACE", "0") == "1"
    res = run_bass_kernel_spmd(
        nc, in_maps, core_ids=list(range(N_CORES)), trace=trace
    )
    if trace:
        print(
            f"HW exec time: {res.exec_time_ns} ns "
            f"(mean {res.mean_exec_time_ns}, max core {res.max_exec_time_core_id})"
        )
    out = np.stack([res.results[b]["out"] for b in range(N_CORES)], axis=0)
    return out
